# revision 24
# baseline (speedup 1.0000x reference)
"""Trainium2 Bass kernel: multi-head attention with decomposed (rel_h + rel_w)
relative position bias.

Shapes (hardcoded): hidden_states (4, 32, 32, 768), NH=12, HD=64.

Sharding: sequence-parallel within each batch. Core c handles batch c//2 and
query rows [hp*512, hp*512+512) with hp = c%2, for ALL 12 heads. K and V are
computed fully (redundantly) by both cores of a pair, so every core owns its
512 output rows completely and no collective is needed.

Per-core trick inventory:
  - xT columns are host-permuted so the core's own 512 query columns come
    first; the selector table (em) is permuted identically, which keeps the
    program SPMD (no core-dependent constants). Attention is invariant to a
    consistent permutation of the key axis.
  - rel_h is host-shifted by hp*16 so the on-device gather offsets are
    core-independent.
  - relative bias is injected into the S^T matmul via basis rows: qh rows
    64..127 hold gathered G tables (G = q . rel), kh rows 64..127 hold a
    one-hot selector; contraction over all 128 rows yields q.k/8 + bias.
  - G tables bounce through DRAM (PSUM -> DRAM -> 3D-affine gather DMA)
    because the diagonal (shear) gather is only expressible on a flat tensor.
"""

import numpy as np
import ml_dtypes

import concourse.bass as bass
import concourse.bacc as bacc
import concourse.mybir as mybir
import concourse.tile as tile
from concourse.bass_utils import run_bass_kernel_spmd

B, HS, WS, C = 4, 32, 32, 768
NH, HD = 12, 64
HW = HS * WS          # 1024
N_CORES = 8
QS = HW // 2          # 512 query rows per core
QH = HS // 2          # 16 query h-rows per core
KT = C // 128         # 6 contraction tiles
NQS = NH * QS         # 6144
NHW = NH * HW         # 12288
F32 = mybir.dt.float32
F32R = mybir.dt.float32r
BF16 = mybir.dt.bfloat16
BF16NP = ml_dtypes.bfloat16


LOGITS_BF16 = True
O2_PSUM = True


def build_program(loop_n=1, logits_bf16=None, o2=None):
    if logits_bf16 is None:
        logits_bf16 = LOGITS_BF16
    if o2 is None:
        o2 = O2_PSUM
    nc = bacc.Bacc("TRN2", target_bir_lowering=False, debug=False,
                   num_devices=N_CORES)

    xT = nc.dram_tensor("xT", [C, HW], BF16, kind="ExternalInput").ap()
    Wq = nc.dram_tensor("Wq", [C, C], BF16, kind="ExternalInput").ap()
    Wk = nc.dram_tensor("Wk", [C, C], BF16, kind="ExternalInput").ap()
    Wv = nc.dram_tensor("Wv", [C, C], BF16, kind="ExternalInput").ap()
    Wp = nc.dram_tensor("Wp", [C, C], BF16, kind="ExternalInput").ap()
    bqk = nc.dram_tensor("bqk", [C, 2], F32, kind="ExternalInput").ap()
    bvr = nc.dram_tensor("bvr", [1, C], F32, kind="ExternalInput").ap()
    bp2 = nc.dram_tensor("bp2", [1, C], F32, kind="ExternalInput").ap()
    LDT = BF16 if logits_bf16 else F32R
    rel = nc.dram_tensor("rel", [64, 111], LDT, kind="ExternalInput").ap()
    em = nc.dram_tensor("em", [64, HW], LDT, kind="ExternalInput").ap()
    out = nc.dram_tensor("out", [QS, C], BF16, kind="ExternalOutput").ap()

    with tile.TileContext(nc) as tc:
        if loop_n > 1:
            with tc.For_i(0, loop_n):
                _body(nc, tc, xT, Wq, Wk, Wv, Wp, bqk, bvr, bp2,
                      rel, em, out, LDT, o2)
        else:
            _body(nc, tc, xT, Wq, Wk, Wv, Wp, bqk, bvr, bp2,
                  rel, em, out, LDT, o2)
    nc.compile()
    return nc


def _body(nc, tc, xT, Wq, Wk, Wv, Wp, bqk, bvr, bp2, rel, em, out,
          LDT, o2=False):
    AF = mybir.ActivationFunctionType
    ALU = mybir.AluOpType

    with (
        tc.tile_pool(name="const", bufs=1) as cpool,
        tc.tile_pool(name="work", bufs=1) as wpool,
        tc.tile_pool(name="gdram", bufs=1, space="DRAM") as gdram,
    ):
        rel_sb = cpool.tile([64, 111], LDT, tag="rel", name="rel")
        nc.scalar.dma_start(rel_sb[:], rel[:])
        relh_sb = rel_sb[:, 0:48]
        relw_sb = rel_sb[:, 48:111]
        bvr_sb = cpool.tile([128, C], F32, tag="bvr", name="bvr")

        # per-head stacked tensors:
        #   qh_all rows: 0-63 qs^T, 64-95 bh basis, 96-127 bw basis
        #   kh_all rows: 0-63 k^T,  64-127 selector (em)
        qh_all = wpool.tile([128, NQS], LDT, tag="qh", name="qh")
        kh_all = wpool.tile([128, NHW], LDT, tag="kh", name="kh")
        V_sb = [wpool.tile([128, NH * 65], BF16, tag=f"v{st}", name=f"v{st}")
                for st in range(8)]
        outT_sb = [wpool.tile([128, QS], BF16, tag=f"oT{p}", name=f"oT{p}")
                   for p in range(6)]


        with tc.tile_pool(name="ph1", bufs=1) as ph1:
            bvr_row = ph1.tile([1, C], F32, tag="bvr_row", name="bvr_row")
            nc.gpsimd.dma_start(bvr_row[:], bvr[:])
            nc.gpsimd.partition_broadcast(bvr_sb[:], bvr_row[0:1, :])
            # selector rows (shared across q; columns follow the host
            # k-perm). One DMA for head 0, then on-chip Pool copies for the
            # other 11 heads (saves ~2.8MB of DMA; the copies sit behind the
            # bvr broadcast so V-assembly is never blocked).
            nc.gpsimd.dma_start(kh_all[64:128, 0:HW], em[0:64, :])
            for n in range(1, NH):
                nc.gpsimd.tensor_copy(kh_all[64:128, n * HW:(n + 1) * HW],
                                      kh_all[64:128, 0:HW])
            xT_sb, Wv_sb, Wq_sb, Wk_sb = [], [], [], []
            for kt in range(KT):
                t = ph1.tile([128, HW], BF16, tag=f"xT{kt}", name=f"xT{kt}")
                nc.sync.dma_start(t[:], xT[kt * 128:(kt + 1) * 128, :])
                xT_sb.append(t)
                t = ph1.tile([128, C], BF16, tag=f"wv{kt}", name=f"wv{kt}")
                nc.sync.dma_start(t[:], Wv[kt * 128:(kt + 1) * 128, :])
                Wv_sb.append(t)
            bqk_sb = ph1.tile([128, 12], F32, tag="bqk", name="bqk")
            nc.scalar.dma_start(
                bqk_sb[:].rearrange("p (a j) -> p a j", j=2),
                bass.AP(tensor=bqk[:].tensor, offset=0,
                        ap=[[2, 128], [256, 6], [1, 2]]))
            for kt in range(KT):
                t = ph1.tile([128, C], BF16, tag=f"wq{kt}", name=f"wq{kt}")
                nc.sync.dma_start(t[:], Wq[kt * 128:(kt + 1) * 128, :])
                Wq_sb.append(t)
            for kt in range(KT):
                t = ph1.tile([128, C], BF16, tag=f"wk{kt}", name=f"wk{kt}")
                nc.gpsimd.dma_start(t[:], Wk[kt * 128:(kt + 1) * 128, :])
                Wk_sb.append(t)

            # ---- phase 1a: V (kt-outer so PE starts after ~0.5MB of DMA),
            # two groups of 4 row-tiles to fit PSUM ----
            with tc.tile_pool(name="ps_v", bufs=1, space="PSUM") as pv:
                for g in range(2):
                    v_ps = [pv.tile([128, C], F32, tag=f"v_ps{i}",
                                    name=f"v_ps{g}{i}") for i in range(4)]
                    for kt in range(KT):
                        for i in range(4):
                            st = g * 4 + i
                            for c0, c1 in ((0, 512), (512, 768)):
                                nc.tensor.matmul(
                                    v_ps[i][:, c0:c1],
                                    xT_sb[kt][:, st * 128:(st + 1) * 128],
                                    Wv_sb[kt][:, c0:c1],
                                    start=(kt == 0), stop=(kt == KT - 1))
                    for i in range(4):
                        st = g * 4 + i
                        nc.vector.tensor_tensor(
                            V_sb[st][:].rearrange("p (n c) -> p n c",
                                                  c=65)[:, :, 0:64],
                            v_ps[i][:].rearrange("p (n c) -> p n c", c=64),
                            bvr_sb[:].rearrange("p (n c) -> p n c", c=64),
                            ALU.add)
                        nc.vector.memset(
                            V_sb[st][:].rearrange("p (n c) -> p n c",
                                                  c=65)[:, :, 64:65],
                            1.0)

            with (
                tc.tile_pool(name="ps_qk", bufs=(1 if o2 else 2),
                             space="PSUM") as pq,
                tc.tile_pool(name="ps_g", bufs=1, space="PSUM") as pg,
                tc.tile_pool(name="gst", bufs=4) as gst,
                tc.tile_pool(name="ps_att", bufs=1, space="PSUM") as pa,
                tc.tile_pool(name="pu", bufs=10) as pu_pool,
                tc.tile_pool(name="rec", bufs=4) as rec_pool,
            ):
                gh_dr = gdram.tile([48, NQS], LDT, tag="gh_dr",
                                   name="gh_dr")
                gw_dr = gdram.tile([63, NQS], LDT, tag="gw_dr",
                                   name="gw_dr")
                bwst = cpool.tile([32, NQS], LDT, tag="bwst", name="bwst")
                # Merged per-head-pair loop: Q proj -> G tables -> DRAM
                # bounce -> K proj -> un-permute -> attention for both heads.
                # Head 0's attention starts ~25us earlier than with
                # phase-sequential ordering; Act (exp) and PE stay balanced.
                for p in range(6):
                    he, ho = 2 * p, 2 * p + 1
                    q_ps = pq.tile([128, QS], F32, tag="qk_ps", name="q_ps")
                    for kt in range(KT):
                        nc.tensor.matmul(
                            q_ps[:],
                            Wq_sb[kt][:, p * 128:(p + 1) * 128],
                            xT_sb[kt][:, 0:QS],
                            start=(kt == 0), stop=(kt == KT - 1))
                    # qs = q/8 + bq/8 (bqk col 2p holds bq/8) on Act
                    nc.scalar.activation(
                        qh_all[0:64, he * QS:(he + 1) * QS],
                        q_ps[0:64, :], AF.Identity,
                        bias=bqk_sb[0:64, 2 * p:2 * p + 1], scale=0.125)
                    nc.scalar.activation(
                        qh_all[0:64, ho * QS:(ho + 1) * QS],
                        q_ps[64:128, :], AF.Identity,
                        bias=bqk_sb[64:128, 2 * p:2 * p + 1], scale=0.125)

                    # G tables + DRAM bounce (SP engine owns every bounce
                    # DMA trigger so compute engines never head-block)
                    for n in (he, ho):
                        nsl = slice(n * QS, (n + 1) * QS)
                        gh_ps = pg.tile([48, QS], F32, tag="g_ps",
                                        name="gh_ps")
                        nc.tensor.matmul(
                            gh_ps[:], relh_sb, qh_all[0:64, nsl],
                            start=True, stop=True, tile_position=(0, 0))
                        gh_sb = gst.tile([48, QS], LDT, tag="gh_sb",
                                         name="gh_sb")
                        nc.scalar.copy(gh_sb[:], gh_ps[:])
                        nc.sync.dma_start(gh_dr[:, nsl], gh_sb[:])
                        gw_ps = pg.tile([63, QS], F32, tag="g_ps",
                                        name="gw_ps")
                        nc.tensor.matmul(
                            gw_ps[:], relw_sb,
                            qh_all[0:64, nsl].rearrange(
                                "p (h w) -> p w h", w=WS),
                            start=True, stop=True, tile_position=(0, 0))
                        gw_sb = gst.tile([63, QS], LDT, tag="gw_sb",
                                         name="gw_sb")
                        nc.vector.tensor_copy(gw_sb[:], gw_ps[:])
                        nc.sync.dma_start(gw_dr[:, nsl], gw_sb[:])
                    # gathers: shear G[h'+r] / G[w+r] via flat-DRAM 3D APs.
                    # bh lands directly in qh rows 64-95; bw lands w-major in
                    # bwst and is un-permuted per head below.
                    for n in (he, ho):
                        nsl = slice(n * QS, (n + 1) * QS)
                        dst_h = qh_all[64:96, nsl].rearrange(
                            "p (h w) -> p h w", w=WS)
                        src_h = bass.AP(tensor=gh_dr[:].tensor,
                                        offset=n * QS,
                                        ap=[[NQS, 32], [NQS + WS, QH],
                                            [1, WS]])
                        nc.sync.dma_start(dst_h, src_h)
                        dst_w = bwst[:, nsl].rearrange(
                            "p (w h) -> p w h", h=QH)
                        src_w = bass.AP(tensor=gw_dr[:].tensor,
                                        offset=n * QS,
                                        ap=[[NQS, 32], [NQS + QH, WS],
                                            [1, QH]])
                        nc.sync.dma_start(dst_w, src_w)

                    # K projection for the pair (fills the gather latency)
                    for sh in range(2):
                        s0 = sh * 512
                        k_ps = pq.tile([128, 512], F32, tag="qk_ps",
                                       name="k_ps")
                        for kt in range(KT):
                            nc.tensor.matmul(
                                k_ps[:],
                                Wk_sb[kt][:, p * 128:(p + 1) * 128],
                                xT_sb[kt][:, s0:s0 + 512],
                                start=(kt == 0), stop=(kt == KT - 1))
                        nc.vector.tensor_scalar_add(
                            kh_all[0:64, he * HW + s0:he * HW + s0 + 512],
                            k_ps[0:64, :], bqk_sb[0:64, 2 * p + 1:2 * p + 2])
                        nc.vector.tensor_scalar_add(
                            kh_all[0:64, ho * HW + s0:ho * HW + s0 + 512],
                            k_ps[64:128, :], bqk_sb[64:128,
                                                    2 * p + 1:2 * p + 2])

                    for n in (he, ho):
                        nsl = slice(n * QS, (n + 1) * QS)
                        eng = nc.vector
                        eng.tensor_copy(
                            qh_all[96:128, nsl].rearrange(
                                "p (h w) -> p h w", w=WS),
                            bwst[:, nsl].rearrange("p (w h) -> p h w", h=QH))

                    # ---- attention for both heads of the pair ----
                    for n in (he, ho):
                        pu_tiles = []
                        for ktp in range(4):  # two k-tiles per psum tile
                            s_ps = pa.tile([128, 1024], F32, tag="s_ps",
                                           name="s_ps", bufs=2)
                            for j in range(2):
                                kt = 2 * ktp + j
                                nc.tensor.matmul(
                                    s_ps[:, j * 512:(j + 1) * 512],
                                    kh_all[:, n * HW + kt * 128:
                                           n * HW + (kt + 1) * 128],
                                    qh_all[:, n * QS:(n + 1) * QS],
                                    start=True, stop=True)
                            pu = pu_pool.tile([128, 1024], BF16, tag="pu",
                                              name="pu")
                            nc.scalar.activation(pu[:], s_ps[:], AF.Exp)
                            pu_tiles.append(pu)
                        o_ps = pa.tile([65, QS], F32, tag="o_ps",
                                       name="o_ps", bufs=(2 if o2 else 1))
                        for ktp in range(4):
                            for j in range(2):
                                kt = 2 * ktp + j
                                nc.tensor.matmul(
                                    o_ps[:],
                                    V_sb[kt][:, n * 65:n * 65 + 65],
                                    pu_tiles[ktp][:, j * 512:(j + 1) * 512],
                                    start=(kt == 0), stop=(kt == 7))
                        rec = rec_pool.tile([1, QS], F32, tag="rec",
                                            name="rec")
                        nc.vector.reciprocal(rec[:], o_ps[64:65, :])
                        rec_bc = rec_pool.tile([64, QS], F32, tag="rec_bc",
                                               name="rec_bc")
                        nc.gpsimd.partition_broadcast(rec_bc[:], rec[0:1, :])
                        nc.vector.tensor_tensor(
                            outT_sb[p][(n % 2) * 64:(n % 2 + 1) * 64, :],
                            o_ps[0:64, :],
                            rec_bc[:],
                            ALU.mult)

        # late constants for phase 4 (scalar queue, off the critical path)
        Wp_sb = []
        for p in range(6):
            t = cpool.tile([128, C], BF16, tag=f"wp{p}", name=f"wp{p}")
            nc.scalar.dma_start(t[:], Wp[p * 128:(p + 1) * 128, :])
            Wp_sb.append(t)
        bp_sb = cpool.tile([128, C], F32, tag="bp", name="bp")

        # ---- phase 4: output projection (+ bp) ----
        with (
            tc.tile_pool(name="ps_pr", bufs=2, space="PSUM") as pp_,
            tc.tile_pool(name="orow", bufs=2) as opool,
        ):
            bp_row = opool.tile([1, C], F32, tag="bp_row", name="bp_row")
            nc.gpsimd.dma_start(bp_row[:], bp2[:])
            nc.gpsimd.partition_broadcast(bp_sb[:], bp_row[0:1, :])
            for qt in range(4):
                qsl = slice(qt * 128, (qt + 1) * 128)
                pr = pp_.tile([128, C], F32, tag="pr", name="pr")
                for p in range(6):
                    for c0, c1 in ((0, 512), (512, 768)):
                        nc.tensor.matmul(
                            pr[:, c0:c1],
                            outT_sb[p][:, qsl],
                            Wp_sb[p][:, c0:c1],
                            start=(p == 0), stop=(p == 5))
                orow = opool.tile([128, C], BF16, tag="orow", name="orow")
                nc.vector.tensor_tensor(orow[:], pr[:], bp_sb[:], ALU.add)
                nc.sync.dma_start(out[qsl, :], orow[:])


def shard_inputs(hidden_states, Wq, bq, Wk, bk, Wv, bv, Wp, bp, rel_h, rel_w):
    """Build the 8 per-core input maps (host-side data movement only)."""
    f = np.float32
    em = np.zeros((64, HW), dtype=f)
    kk = np.arange(HW)
    em[31 - kk // WS, kk] = 1.0
    em[32 + 31 - kk % WS, kk] = 1.0
    rh8 = np.ascontiguousarray(8.0 * np.asarray(rel_h).astype(f).T)  # [64,63]
    rw8 = np.ascontiguousarray(8.0 * np.asarray(rel_w).astype(f).T)  # [64,63]

    wq = np.asarray(Wq).astype(f).astype(BF16NP)
    wk = np.asarray(Wk).astype(f).astype(BF16NP)
    wv = np.asarray(Wv).astype(f).astype(BF16NP)
    wp = np.asarray(Wp).astype(f).astype(BF16NP)
    bqk = np.ascontiguousarray(np.stack(
        [np.asarray(bq).astype(f) / 8.0, np.asarray(bk).astype(f)],
        axis=1))  # [C, 2]
    bvr = np.ascontiguousarray(np.asarray(bv).reshape(1, C).astype(f))
    bp2 = np.ascontiguousarray(np.asarray(bp).reshape(1, C).astype(f))

    in_maps = []
    for c in range(N_CORES):
        b, hp = c // 2, c % 2
        xTb = np.asarray(hidden_states)[b].reshape(HW, C).T.astype(f)
        perm = np.r_[hp * QS:(hp + 1) * QS, (1 - hp) * QS:(2 - hp) * QS]
        rel_c = np.zeros((64, 111), dtype=f)
        wdt = min(63 - hp * QH, 48)
        rel_c[:, :wdt] = rh8[:, hp * QH:hp * QH + wdt]
        rel_c[:, 48:111] = rw8
        ldt = BF16NP if LOGITS_BF16 else np.float32
        in_maps.append({
            "xT": np.ascontiguousarray(xTb[:, perm]).astype(BF16NP),
            "Wq": wq, "Wk": wk, "Wv": wv, "Wp": wp,
            "bqk": bqk, "bvr": bvr, "bp2": bp2,
            "rel": rel_c.astype(ldt),
            "em": np.ascontiguousarray(em[:, perm]).astype(ldt),
        })
    return in_maps


_NC_CACHE = {}


def get_program(loop_n=1):
    if loop_n not in _NC_CACHE:
        _NC_CACHE[loop_n] = build_program(loop_n=loop_n)
    return _NC_CACHE[loop_n]


def kernel(hidden_states, Wq, bq, Wk, bk, Wv, bv, Wp, bp, rel_h, rel_w):
    in_maps = shard_inputs(hidden_states, Wq, bq, Wk, bk, Wv, bv, Wp, bp,
                           rel_h, rel_w)
    nc = get_program()
    res = run_bass_kernel_spmd(nc, in_maps, list(range(N_CORES)))
    full = np.empty((B, HS, WS, C), dtype=np.float32)
    fr = full.reshape(B, HW, C)
    for c in range(N_CORES):
        b, hp = c // 2, c % 2
        fr[b, hp * QS:(hp + 1) * QS] = res.results[c]["out"].astype(
            np.float32)
    return full


# revision 26
# speedup vs baseline: 1.0139x; 1.0139x over previous
"""Trainium2 Bass kernel: multi-head attention with decomposed (rel_h + rel_w)
relative position bias.

Shapes (hardcoded): hidden_states (4, 32, 32, 768), NH=12, HD=64.

Sharding: sequence-parallel within each batch. Core c handles batch c//2 and
query rows [hp*512, hp*512+512) with hp = c%2, for ALL 12 heads. K and V are
computed fully (redundantly) by both cores of a pair, so every core owns its
512 output rows completely and no collective is needed.

Per-core trick inventory:
  - xT columns are host-permuted so the core's own 512 query columns come
    first; the selector table (em) is permuted identically, which keeps the
    program SPMD (no core-dependent constants). Attention is invariant to a
    consistent permutation of the key axis.
  - rel_h is host-shifted by hp*16 so the on-device gather offsets are
    core-independent.
  - relative bias is injected into the S^T matmul via basis rows: qh rows
    64..127 hold gathered G tables (G = q . rel), kh rows 64..127 hold a
    one-hot selector; contraction over all 128 rows yields q.k/8 + bias.
  - G tables bounce through DRAM (PSUM -> DRAM -> 3D-affine gather DMA)
    because the diagonal (shear) gather is only expressible on a flat tensor.
"""

import numpy as np
import ml_dtypes

import concourse.bass as bass
import concourse.bacc as bacc
import concourse.mybir as mybir
import concourse.tile as tile
from concourse.bass_utils import run_bass_kernel_spmd

B, HS, WS, C = 4, 32, 32, 768
NH, HD = 12, 64
HW = HS * WS          # 1024
N_CORES = 8
QS = HW // 2          # 512 query rows per core
QH = HS // 2          # 16 query h-rows per core
KT = C // 128         # 6 contraction tiles
NQS = NH * QS         # 6144
NHW = NH * HW         # 12288
F32 = mybir.dt.float32
F32R = mybir.dt.float32r
BF16 = mybir.dt.bfloat16
BF16NP = ml_dtypes.bfloat16


LOGITS_BF16 = True
O2_PSUM = True


def build_program(loop_n=1, logits_bf16=None, o2=None):
    if logits_bf16 is None:
        logits_bf16 = LOGITS_BF16
    if o2 is None:
        o2 = O2_PSUM
    nc = bacc.Bacc("TRN2", target_bir_lowering=False, debug=False,
                   num_devices=N_CORES)

    xT = nc.dram_tensor("xT", [C, HW], BF16, kind="ExternalInput").ap()
    Wq = nc.dram_tensor("Wq", [C, C], BF16, kind="ExternalInput").ap()
    Wk = nc.dram_tensor("Wk", [C, C], BF16, kind="ExternalInput").ap()
    Wv = nc.dram_tensor("Wv", [C, C], BF16, kind="ExternalInput").ap()
    Wp = nc.dram_tensor("Wp", [C, C], BF16, kind="ExternalInput").ap()
    bqk = nc.dram_tensor("bqk", [C, 2], F32, kind="ExternalInput").ap()
    bvr = nc.dram_tensor("bvr", [1, C], F32, kind="ExternalInput").ap()
    bp2 = nc.dram_tensor("bp2", [1, C], F32, kind="ExternalInput").ap()
    LDT = BF16 if logits_bf16 else F32R
    rel = nc.dram_tensor("rel", [64, 111], LDT, kind="ExternalInput").ap()
    em = nc.dram_tensor("em", [64, HW], LDT, kind="ExternalInput").ap()
    out = nc.dram_tensor("out", [QS, C], BF16, kind="ExternalOutput").ap()

    with tile.TileContext(nc) as tc:
        if loop_n > 1:
            with tc.For_i(0, loop_n):
                _body(nc, tc, xT, Wq, Wk, Wv, Wp, bqk, bvr, bp2,
                      rel, em, out, LDT, o2)
        else:
            _body(nc, tc, xT, Wq, Wk, Wv, Wp, bqk, bvr, bp2,
                  rel, em, out, LDT, o2)
    nc.compile()
    return nc


def _body(nc, tc, xT, Wq, Wk, Wv, Wp, bqk, bvr, bp2, rel, em, out,
          LDT, o2=False):
    AF = mybir.ActivationFunctionType
    ALU = mybir.AluOpType

    with (
        tc.tile_pool(name="const", bufs=1) as cpool,
        tc.tile_pool(name="work", bufs=1) as wpool,
        tc.tile_pool(name="gdram", bufs=1, space="DRAM") as gdram,
    ):
        rel_sb = cpool.tile([64, 111], LDT, tag="rel", name="rel")
        nc.scalar.dma_start(rel_sb[:], rel[:])
        relh_sb = rel_sb[:, 0:48]
        relw_sb = rel_sb[:, 48:111]
        bvr_sb = cpool.tile([128, C], F32, tag="bvr", name="bvr")

        # per-head stacked tensors:
        #   qh_all rows: 0-63 qs^T, 64-95 bh basis, 96-127 bw basis
        #   kh_all rows: 0-63 k^T,  64-127 selector (em)
        qh_all = wpool.tile([128, NQS], LDT, tag="qh", name="qh")
        kh_all = wpool.tile([128, NHW], LDT, tag="kh", name="kh")
        V_sb = [wpool.tile([128, NH * 65], BF16, tag=f"v{st}", name=f"v{st}")
                for st in range(8)]
        outT_sb = [wpool.tile([128, QS], BF16, tag=f"oT{p}", name=f"oT{p}")
                   for p in range(6)]


        with tc.tile_pool(name="ph1", bufs=1) as ph1:
            bvr_row = ph1.tile([1, C], F32, tag="bvr_row", name="bvr_row")
            nc.gpsimd.dma_start(bvr_row[:], bvr[:])
            nc.gpsimd.partition_broadcast(bvr_sb[:], bvr_row[0:1, :])
            # selector rows (shared across q; columns follow the host
            # k-perm). One DMA for head 0, then on-chip Pool copies for the
            # other 11 heads (saves ~2.8MB of DMA; the copies sit behind the
            # bvr broadcast so V-assembly is never blocked).
            nc.gpsimd.dma_start(kh_all[64:128, 0:HW], em[0:64, :])
            for n in range(1, NH):
                nc.gpsimd.tensor_copy(kh_all[64:128, n * HW:(n + 1) * HW],
                                      kh_all[64:128, 0:HW])
            xT_sb, Wv_sb, Wq_sb, Wk_sb = [], [], [], []
            for kt in range(KT):
                t = ph1.tile([128, HW], BF16, tag=f"xT{kt}", name=f"xT{kt}")
                nc.sync.dma_start(t[:], xT[kt * 128:(kt + 1) * 128, :])
                xT_sb.append(t)
                t = ph1.tile([128, C], BF16, tag=f"wv{kt}", name=f"wv{kt}")
                nc.sync.dma_start(t[:], Wv[kt * 128:(kt + 1) * 128, :])
                Wv_sb.append(t)
            bqk_sb = ph1.tile([128, 12], F32, tag="bqk", name="bqk")
            nc.scalar.dma_start(
                bqk_sb[:].rearrange("p (a j) -> p a j", j=2),
                bass.AP(tensor=bqk[:].tensor, offset=0,
                        ap=[[2, 128], [256, 6], [1, 2]]))
            for kt in range(KT):
                t = ph1.tile([128, C], BF16, tag=f"wq{kt}", name=f"wq{kt}")
                nc.sync.dma_start(t[:], Wq[kt * 128:(kt + 1) * 128, :])
                Wq_sb.append(t)
            for kt in range(KT):
                t = ph1.tile([128, C], BF16, tag=f"wk{kt}", name=f"wk{kt}")
                nc.gpsimd.dma_start(t[:], Wk[kt * 128:(kt + 1) * 128, :])
                Wk_sb.append(t)

            # ---- phase 1a: V (kt-outer so PE starts after ~0.5MB of DMA),
            # two groups of 4 row-tiles to fit PSUM ----
            with tc.tile_pool(name="ps_v", bufs=1, space="PSUM") as pv:
                for g in range(2):
                    v_ps = [pv.tile([128, C], F32, tag=f"v_ps{i}",
                                    name=f"v_ps{g}{i}") for i in range(4)]
                    for kt in range(KT):
                        for i in range(4):
                            st = g * 4 + i
                            for c0, c1 in ((0, 512), (512, 768)):
                                nc.tensor.matmul(
                                    v_ps[i][:, c0:c1],
                                    xT_sb[kt][:, st * 128:(st + 1) * 128],
                                    Wv_sb[kt][:, c0:c1],
                                    start=(kt == 0), stop=(kt == KT - 1))
                    for i in range(4):
                        st = g * 4 + i
                        nc.vector.tensor_tensor(
                            V_sb[st][:].rearrange("p (n c) -> p n c",
                                                  c=65)[:, :, 0:64],
                            v_ps[i][:].rearrange("p (n c) -> p n c", c=64),
                            bvr_sb[:].rearrange("p (n c) -> p n c", c=64),
                            ALU.add)
                        nc.vector.memset(
                            V_sb[st][:].rearrange("p (n c) -> p n c",
                                                  c=65)[:, :, 64:65],
                            1.0)

            with (
                tc.tile_pool(name="ps_qk", bufs=(1 if o2 else 2),
                             space="PSUM") as pq,
                tc.tile_pool(name="ps_g", bufs=1, space="PSUM") as pg,
                tc.tile_pool(name="gst", bufs=4) as gst,
                tc.tile_pool(name="ps_att", bufs=1, space="PSUM") as pa,
                tc.tile_pool(name="pu", bufs=10) as pu_pool,
                tc.tile_pool(name="rec", bufs=4) as rec_pool,
            ):
                gh_dr = gdram.tile([48, NQS], LDT, tag="gh_dr",
                                   name="gh_dr")
                gw_dr = gdram.tile([63, NQS], LDT, tag="gw_dr",
                                   name="gw_dr")
                bwst = cpool.tile([32, NQS], LDT, tag="bwst", name="bwst")
                # Merged per-head-pair loop: Q proj -> G tables -> DRAM
                # bounce -> K proj -> un-permute -> attention for both heads.
                # Head 0's attention starts ~25us earlier than with
                # phase-sequential ordering; Act (exp) and PE stay balanced.
                for p in range(6):
                    he, ho = 2 * p, 2 * p + 1
                    q_ps = pq.tile([128, QS], F32, tag="qk_ps", name="q_ps")
                    for kt in range(KT):
                        nc.tensor.matmul(
                            q_ps[:],
                            Wq_sb[kt][:, p * 128:(p + 1) * 128],
                            xT_sb[kt][:, 0:QS],
                            start=(kt == 0), stop=(kt == KT - 1))
                    # qs = q/8 + bq/8 (bqk col 2p holds bq/8) on Act
                    nc.scalar.activation(
                        qh_all[0:64, he * QS:(he + 1) * QS],
                        q_ps[0:64, :], AF.Identity,
                        bias=bqk_sb[0:64, 2 * p:2 * p + 1], scale=0.125)
                    nc.scalar.activation(
                        qh_all[0:64, ho * QS:(ho + 1) * QS],
                        q_ps[64:128, :], AF.Identity,
                        bias=bqk_sb[64:128, 2 * p:2 * p + 1], scale=0.125)

                    # G tables + DRAM bounce (SP engine owns every bounce
                    # DMA trigger so compute engines never head-block)
                    for n in (he, ho):
                        nsl = slice(n * QS, (n + 1) * QS)
                        gh_ps = pg.tile([48, QS], F32, tag="g_ps",
                                        name="gh_ps")
                        nc.tensor.matmul(
                            gh_ps[:], relh_sb, qh_all[0:64, nsl],
                            start=True, stop=True, tile_position=(0, 0))
                        gh_sb = gst.tile([48, QS], LDT, tag="gh_sb",
                                         name="gh_sb")
                        nc.scalar.copy(gh_sb[:], gh_ps[:])
                        nc.sync.dma_start(gh_dr[:, nsl], gh_sb[:])
                        gw_ps = pg.tile([63, QS], F32, tag="g_ps",
                                        name="gw_ps")
                        nc.tensor.matmul(
                            gw_ps[:], relw_sb,
                            qh_all[0:64, nsl].rearrange(
                                "p (h w) -> p w h", w=WS),
                            start=True, stop=True, tile_position=(0, 0))
                        gw_sb = gst.tile([63, QS], LDT, tag="gw_sb",
                                         name="gw_sb")
                        nc.vector.tensor_copy(gw_sb[:], gw_ps[:])
                        nc.sync.dma_start(gw_dr[:, nsl], gw_sb[:])
                    # gathers: shear G[h'+r] / G[w+r] via flat-DRAM 3D APs.
                    # bh lands directly in qh rows 64-95; bw lands w-major in
                    # bwst and is un-permuted per head below.
                    for n in (he, ho):
                        nsl = slice(n * QS, (n + 1) * QS)
                        dst_h = qh_all[64:96, nsl].rearrange(
                            "p (h w) -> p h w", w=WS)
                        src_h = bass.AP(tensor=gh_dr[:].tensor,
                                        offset=n * QS,
                                        ap=[[NQS, 32], [NQS + WS, QH],
                                            [1, WS]])
                        nc.sync.dma_start(dst_h, src_h)
                        dst_w = bwst[:, nsl].rearrange(
                            "p (w h) -> p w h", h=QH)
                        src_w = bass.AP(tensor=gw_dr[:].tensor,
                                        offset=n * QS,
                                        ap=[[NQS, 32], [NQS + QH, WS],
                                            [1, QH]])
                        nc.sync.dma_start(dst_w, src_w)

                    # K projection for the pair (fills the gather latency)
                    for sh in range(2):
                        s0 = sh * 512
                        k_ps = pq.tile([128, 512], F32, tag="qk_ps",
                                       name="k_ps")
                        for kt in range(KT):
                            nc.tensor.matmul(
                                k_ps[:],
                                Wk_sb[kt][:, p * 128:(p + 1) * 128],
                                xT_sb[kt][:, s0:s0 + 512],
                                start=(kt == 0), stop=(kt == KT - 1))
                        nc.vector.tensor_scalar_add(
                            kh_all[0:64, he * HW + s0:he * HW + s0 + 512],
                            k_ps[0:64, :], bqk_sb[0:64, 2 * p + 1:2 * p + 2])
                        nc.vector.tensor_scalar_add(
                            kh_all[0:64, ho * HW + s0:ho * HW + s0 + 512],
                            k_ps[64:128, :], bqk_sb[64:128,
                                                    2 * p + 1:2 * p + 2])

                    for n in (he, ho):
                        nsl = slice(n * QS, (n + 1) * QS)
                        eng = nc.vector
                        eng.tensor_copy(
                            qh_all[96:128, nsl].rearrange(
                                "p (h w) -> p h w", w=WS),
                            bwst[:, nsl].rearrange("p (w h) -> p h w", h=QH))

                    # ---- attention for both heads of the pair ----
                    for n in (he, ho):
                        pu_tiles = []
                        for ktp in range(4):  # two k-tiles per psum tile
                            s_ps = pa.tile([128, 1024], F32, tag="s_ps",
                                           name="s_ps", bufs=2)
                            for j in range(2):
                                kt = 2 * ktp + j
                                nc.tensor.matmul(
                                    s_ps[:, j * 512:(j + 1) * 512],
                                    kh_all[:, n * HW + kt * 128:
                                           n * HW + (kt + 1) * 128],
                                    qh_all[:, n * QS:(n + 1) * QS],
                                    start=True, stop=True)
                            pu = pu_pool.tile([128, 1024], BF16, tag="pu",
                                              name="pu")
                            nc.scalar.activation(pu[:], s_ps[:], AF.Exp)
                            pu_tiles.append(pu)
                        o_ps = pa.tile([65, QS], F32, tag="o_ps",
                                       name="o_ps", bufs=(2 if o2 else 1))
                        for ktp in range(4):
                            for j in range(2):
                                kt = 2 * ktp + j
                                nc.tensor.matmul(
                                    o_ps[:],
                                    V_sb[kt][:, n * 65:n * 65 + 65],
                                    pu_tiles[ktp][:, j * 512:(j + 1) * 512],
                                    start=(kt == 0), stop=(kt == 7))
                        rec = rec_pool.tile([1, QS], F32, tag="rec",
                                            name="rec")
                        nc.vector.reciprocal(rec[:], o_ps[64:65, :])
                        rec_bc = rec_pool.tile([64, QS], F32, tag="rec_bc",
                                               name="rec_bc")
                        nc.gpsimd.partition_broadcast(rec_bc[:], rec[0:1, :])
                        nc.vector.tensor_tensor(
                            outT_sb[p][(n % 2) * 64:(n % 2 + 1) * 64, :],
                            o_ps[0:64, :],
                            rec_bc[:],
                            ALU.mult)

        # late constants for phase 4 (scalar queue, off the critical path)
        Wp_sb = []
        for p in range(6):
            t = cpool.tile([128, C], BF16, tag=f"wp{p}", name=f"wp{p}")
            nc.scalar.dma_start(t[:], Wp[p * 128:(p + 1) * 128, :])
            Wp_sb.append(t)
        bp_sb = cpool.tile([128, C], F32, tag="bp", name="bp")

        # ---- phase 4: output projection (+ bp) ----
        with (
            tc.tile_pool(name="ps_pr", bufs=2, space="PSUM") as pp_,
            tc.tile_pool(name="orow", bufs=2) as opool,
        ):
            bp_row = opool.tile([1, C], F32, tag="bp_row", name="bp_row")
            nc.gpsimd.dma_start(bp_row[:], bp2[:])
            nc.gpsimd.partition_broadcast(bp_sb[:], bp_row[0:1, :])
            for qt in range(4):
                qsl = slice(qt * 128, (qt + 1) * 128)
                pr = pp_.tile([128, C], F32, tag="pr", name="pr")
                for p in range(6):
                    for c0, c1 in ((0, 512), (512, 768)):
                        nc.tensor.matmul(
                            pr[:, c0:c1],
                            outT_sb[p][:, qsl],
                            Wp_sb[p][:, c0:c1],
                            start=(p == 0), stop=(p == 5))
                orow = opool.tile([128, C], BF16, tag="orow", name="orow")
                nc.vector.tensor_tensor(orow[:], pr[:], bp_sb[:], ALU.add)
                nc.sync.dma_start(out[qsl, :], orow[:])


def shard_inputs(hidden_states, Wq, bq, Wk, bk, Wv, bv, Wp, bp, rel_h, rel_w):
    """Build the 8 per-core input maps (host-side data movement only)."""
    f = np.float32
    em = np.zeros((64, HW), dtype=f)
    kk = np.arange(HW)
    em[31 - kk // WS, kk] = 1.0
    em[32 + 31 - kk % WS, kk] = 1.0
    rh8 = np.ascontiguousarray(8.0 * np.asarray(rel_h).astype(f).T)  # [64,63]
    rw8 = np.ascontiguousarray(8.0 * np.asarray(rel_w).astype(f).T)  # [64,63]

    wq = np.asarray(Wq).astype(f).astype(BF16NP)
    wk = np.asarray(Wk).astype(f).astype(BF16NP)
    wv = np.asarray(Wv).astype(f).astype(BF16NP)
    wp = np.asarray(Wp).astype(f).astype(BF16NP)
    bqk = np.ascontiguousarray(np.stack(
        [np.asarray(bq).astype(f) / 8.0, np.asarray(bk).astype(f)],
        axis=1))  # [C, 2]
    bvr = np.ascontiguousarray(np.asarray(bv).reshape(1, C).astype(f))
    bp2 = np.ascontiguousarray(np.asarray(bp).reshape(1, C).astype(f))

    in_maps = []
    for c in range(N_CORES):
        b, hp = c // 2, c % 2
        xTb = np.asarray(hidden_states)[b].reshape(HW, C).T.astype(f)
        perm = np.r_[hp * QS:(hp + 1) * QS, (1 - hp) * QS:(2 - hp) * QS]
        rel_c = np.zeros((64, 111), dtype=f)
        wdt = min(63 - hp * QH, 48)
        rel_c[:, :wdt] = rh8[:, hp * QH:hp * QH + wdt]
        rel_c[:, 48:111] = rw8
        ldt = BF16NP if LOGITS_BF16 else np.float32
        in_maps.append({
            "xT": np.ascontiguousarray(xTb[:, perm]).astype(BF16NP),
            "Wq": wq, "Wk": wk, "Wv": wv, "Wp": wp,
            "bqk": bqk, "bvr": bvr, "bp2": bp2,
            "rel": rel_c.astype(ldt),
            "em": np.ascontiguousarray(em[:, perm]).astype(ldt),
        })
    return in_maps


_NC_CACHE = {}


def get_program(loop_n=1):
    if loop_n not in _NC_CACHE:
        _NC_CACHE[loop_n] = build_program(loop_n=loop_n)
    return _NC_CACHE[loop_n]


def kernel(hidden_states, Wq, bq, Wk, bk, Wv, bv, Wp, bp, rel_h, rel_w):
    in_maps = shard_inputs(hidden_states, Wq, bq, Wk, bk, Wv, bv, Wp, bp,
                           rel_h, rel_w)
    nc = get_program()
    res = run_bass_kernel_spmd(nc, in_maps, list(range(N_CORES)))
    full = np.empty((B, HS, WS, C), dtype=np.float32)
    fr = full.reshape(B, HW, C)
    for c in range(N_CORES):
        b, hp = c // 2, c % 2
        fr[b, hp * QS:(hp + 1) * QS] = res.results[c]["out"].astype(
            np.float32)
    return full


# revision 34
# speedup vs baseline: 1.0152x; 1.0013x over previous
"""Trainium2 Bass kernel: multi-head attention with decomposed (rel_h + rel_w)
relative position bias.

Shapes (hardcoded): hidden_states (4, 32, 32, 768), NH=12, HD=64.

Sharding: sequence-parallel within each batch. Core c handles batch c//2 and
query rows [hp*512, hp*512+512) with hp = c%2, for ALL 12 heads. K and V are
computed fully (redundantly) by both cores of a pair, so every core owns its
512 output rows completely and no collective is needed.

Per-core trick inventory:
  - xT columns are host-permuted so the core's own 512 query columns come
    first; the selector table (em) is permuted identically, which keeps the
    program SPMD (no core-dependent constants). Attention is invariant to a
    consistent permutation of the key axis.
  - rel_h is host-shifted by hp*16 so the on-device gather offsets are
    core-independent.
  - relative bias is injected into the S^T matmul via basis rows: qh rows
    64..127 hold gathered G tables (G = q . rel), kh rows 64..127 hold a
    one-hot selector; contraction over all 128 rows yields q.k/8 + bias.
  - G tables bounce through DRAM (PSUM -> DRAM -> 3D-affine gather DMA)
    because the diagonal (shear) gather is only expressible on a flat tensor.
"""

import numpy as np
import ml_dtypes

import concourse.bass as bass
import concourse.bacc as bacc
import concourse.mybir as mybir
import concourse.tile as tile
from concourse.bass_utils import run_bass_kernel_spmd

B, HS, WS, C = 4, 32, 32, 768
NH, HD = 12, 64
HW = HS * WS          # 1024
N_CORES = 8
QS = HW // 2          # 512 query rows per core
QH = HS // 2          # 16 query h-rows per core
KT = C // 128         # 6 contraction tiles
NQS = NH * QS         # 6144
NHW = NH * HW         # 12288
F32 = mybir.dt.float32
F32R = mybir.dt.float32r
BF16 = mybir.dt.bfloat16
BF16NP = ml_dtypes.bfloat16


LOGITS_BF16 = True
O2_PSUM = True


def build_program(loop_n=1, logits_bf16=None, o2=None):
    if logits_bf16 is None:
        logits_bf16 = LOGITS_BF16
    if o2 is None:
        o2 = O2_PSUM
    nc = bacc.Bacc("TRN2", target_bir_lowering=False, debug=False,
                   num_devices=N_CORES)

    xT = nc.dram_tensor("xT", [C, HW], BF16, kind="ExternalInput").ap()
    Wq = nc.dram_tensor("Wq", [C, C], BF16, kind="ExternalInput").ap()
    Wk = nc.dram_tensor("Wk", [C, C], BF16, kind="ExternalInput").ap()
    Wv = nc.dram_tensor("Wv", [C, C], BF16, kind="ExternalInput").ap()
    Wp = nc.dram_tensor("Wp", [C, C], BF16, kind="ExternalInput").ap()
    bqk = nc.dram_tensor("bqk", [C, 2], F32, kind="ExternalInput").ap()
    bvr = nc.dram_tensor("bvr", [1, C], F32, kind="ExternalInput").ap()
    bp2 = nc.dram_tensor("bp2", [1, C], F32, kind="ExternalInput").ap()
    LDT = BF16 if logits_bf16 else F32R
    rel = nc.dram_tensor("rel", [64, 111], LDT, kind="ExternalInput").ap()
    em = nc.dram_tensor("em", [64, HW], LDT, kind="ExternalInput").ap()
    out = nc.dram_tensor("out", [QS, C], BF16, kind="ExternalOutput").ap()

    with tile.TileContext(nc) as tc:
        if loop_n > 1:
            with tc.For_i(0, loop_n):
                _body(nc, tc, xT, Wq, Wk, Wv, Wp, bqk, bvr, bp2,
                      rel, em, out, LDT, o2)
        else:
            _body(nc, tc, xT, Wq, Wk, Wv, Wp, bqk, bvr, bp2,
                  rel, em, out, LDT, o2)
    nc.compile()
    return nc


def _pair_qg(nc, tc, p, pq, gst, qh_all, bqk_sb, Wq_sb, xT_sb, relh_sb,
             relw_sb, gh_dr, gw_dr, bwst, LDT, AF):
    """Q projection + G tables + DRAM bounce + gathers for head pair p."""
    he, ho = 2 * p, 2 * p + 1
    q_ps = pq.tile([128, QS], F32, tag="qk_ps", name="q_ps")
    for kt in range(KT):
        nc.tensor.matmul(
            q_ps[:],
            Wq_sb[kt][:, p * 128:(p + 1) * 128],
            xT_sb[kt][:, 0:QS],
            start=(kt == 0), stop=(kt == KT - 1))
    # qs = q/8 + bq/8 (bqk col 2p holds bq/8) on Act
    nc.scalar.activation(
        qh_all[0:64, he * QS:(he + 1) * QS],
        q_ps[0:64, :], AF.Identity,
        bias=bqk_sb[0:64, 2 * p:2 * p + 1], scale=0.125)
    nc.scalar.activation(
        qh_all[0:64, ho * QS:(ho + 1) * QS],
        q_ps[64:128, :], AF.Identity,
        bias=bqk_sb[64:128, 2 * p:2 * p + 1], scale=0.125)

    # G tables + DRAM bounce (SP engine owns every bounce DMA trigger so
    # compute engines never head-block)
    for n in (he, ho):
        nsl = slice(n * QS, (n + 1) * QS)
        gh_ps = pq.tile([48, QS], F32, tag="g_ps", name="gh_ps")
        nc.tensor.matmul(
            gh_ps[:], relh_sb, qh_all[0:64, nsl],
            start=True, stop=True, tile_position=(0, 0))
        gh_sb = gst.tile([48, QS], LDT, tag="gh_sb", name="gh_sb")
        nc.scalar.copy(gh_sb[:], gh_ps[:])
        nc.sync.dma_start(gh_dr[:, nsl], gh_sb[:])
        gw_ps = pq.tile([63, QS], F32, tag="g_ps", name="gw_ps")
        nc.tensor.matmul(
            gw_ps[:], relw_sb,
            qh_all[0:64, nsl].rearrange("p (h w) -> p w h", w=WS),
            start=True, stop=True, tile_position=(0, 0))
        gw_sb = gst.tile([63, QS], LDT, tag="gw_sb", name="gw_sb")
        nc.vector.tensor_copy(gw_sb[:], gw_ps[:])
        nc.sync.dma_start(gw_dr[:, nsl], gw_sb[:])
    # gathers: shear G[h'+r] / G[w+r] via flat-DRAM 3D APs. bh lands
    # directly in qh rows 64-95; bw lands w-major in bwst.
    for n in (he, ho):
        nsl = slice(n * QS, (n + 1) * QS)
        dst_h = qh_all[64:96, nsl].rearrange("p (h w) -> p h w", w=WS)
        src_h = bass.AP(tensor=gh_dr[:].tensor, offset=n * QS,
                        ap=[[NQS, 32], [NQS + WS, QH], [1, WS]])
        nc.sync.dma_start(dst_h, src_h)
        dst_w = bwst[:, nsl].rearrange("p (w h) -> p w h", h=QH)
        src_w = bass.AP(tensor=gw_dr[:].tensor, offset=n * QS,
                        ap=[[NQS, 32], [NQS + QH, WS], [1, QH]])
        nc.sync.dma_start(dst_w, src_w)


def _body(nc, tc, xT, Wq, Wk, Wv, Wp, bqk, bvr, bp2, rel, em, out,
          LDT, o2=False):
    AF = mybir.ActivationFunctionType
    ALU = mybir.AluOpType

    with (
        tc.tile_pool(name="const", bufs=1) as cpool,
        tc.tile_pool(name="work", bufs=1) as wpool,
        tc.tile_pool(name="gdram", bufs=1, space="DRAM") as gdram,
    ):
        rel_sb = cpool.tile([64, 111], LDT, tag="rel", name="rel")
        nc.scalar.dma_start(rel_sb[:], rel[:])
        relh_sb = rel_sb[:, 0:48]
        relw_sb = rel_sb[:, 48:111]
        bvr_sb = cpool.tile([128, C], F32, tag="bvr", name="bvr")

        # per-head stacked tensors:
        #   qh_all rows: 0-63 qs^T, 64-95 bh basis, 96-127 bw basis
        #   kh_all rows: 0-63 k^T,  64-127 selector (em)
        qh_all = wpool.tile([128, NQS], LDT, tag="qh", name="qh")
        kh_all = wpool.tile([128, NHW], LDT, tag="kh", name="kh")
        V_sb = [wpool.tile([128, NH * 65], BF16, tag=f"v{st}", name=f"v{st}")
                for st in range(8)]
        outT_sb = [wpool.tile([128, QS], BF16, tag=f"oT{p}", name=f"oT{p}")
                   for p in range(6)]


        with tc.tile_pool(name="ph1", bufs=1) as ph1:
            bvr_row = ph1.tile([1, C], F32, tag="bvr_row", name="bvr_row")
            nc.gpsimd.dma_start(bvr_row[:], bvr[:])
            nc.gpsimd.partition_broadcast(bvr_sb[:], bvr_row[0:1, :])
            # selector rows (shared across q; columns follow the host
            # k-perm). One DMA for head 0, then on-chip Pool copies for the
            # other 11 heads (saves ~2.8MB of DMA; the copies sit behind the
            # bvr broadcast so V-assembly is never blocked).
            nc.gpsimd.dma_start(kh_all[64:128, 0:HW], em[0:64, :])
            for n in range(1, NH):
                nc.gpsimd.tensor_copy(kh_all[64:128, n * HW:(n + 1) * HW],
                                      kh_all[64:128, 0:HW])
            xT_sb, Wv_sb, Wq_sb, Wk_sb = [], [], [], []
            for kt in range(KT):
                t = ph1.tile([128, HW], BF16, tag=f"xT{kt}", name=f"xT{kt}")
                nc.sync.dma_start(t[:], xT[kt * 128:(kt + 1) * 128, :])
                xT_sb.append(t)
                t = ph1.tile([128, C], BF16, tag=f"wv{kt}", name=f"wv{kt}")
                nc.sync.dma_start(t[:], Wv[kt * 128:(kt + 1) * 128, :])
                Wv_sb.append(t)
            bqk_sb = ph1.tile([128, 12], F32, tag="bqk", name="bqk")
            nc.scalar.dma_start(
                bqk_sb[:].rearrange("p (a j) -> p a j", j=2),
                bass.AP(tensor=bqk[:].tensor, offset=0,
                        ap=[[2, 128], [256, 6], [1, 2]]))
            for kt in range(KT):
                t = ph1.tile([128, C], BF16, tag=f"wq{kt}", name=f"wq{kt}")
                nc.sync.dma_start(t[:], Wq[kt * 128:(kt + 1) * 128, :])
                Wq_sb.append(t)
            for kt in range(KT):
                t = ph1.tile([128, C], BF16, tag=f"wk{kt}", name=f"wk{kt}")
                nc.gpsimd.dma_start(t[:], Wk[kt * 128:(kt + 1) * 128, :])
                Wk_sb.append(t)

            # ---- phase 1a: V (kt-outer so PE starts after ~0.5MB of DMA),
            # two groups of 4 row-tiles to fit PSUM ----
            with tc.tile_pool(name="ps_v", bufs=1, space="PSUM") as pv:
                def v_group(sts, g):
                    v_ps = [pv.tile([128, C], F32, tag=f"v_ps{i}",
                                    name=f"v_ps{g}{i}")
                            for i in range(len(sts))]
                    for kt in range(KT):
                        for i, st in enumerate(sts):
                            for c0, c1 in ((0, 512), (512, 768)):
                                nc.tensor.matmul(
                                    v_ps[i][:, c0:c1],
                                    xT_sb[kt][:, st * 128:(st + 1) * 128],
                                    Wv_sb[kt][:, c0:c1],
                                    start=(kt == 0), stop=(kt == KT - 1))
                    for i, st in enumerate(sts):
                        nc.vector.tensor_tensor(
                            V_sb[st][:].rearrange("p (n c) -> p n c",
                                                  c=65)[:, :, 0:64],
                            v_ps[i][:].rearrange("p (n c) -> p n c", c=64),
                            bvr_sb[:].rearrange("p (n c) -> p n c", c=64),
                            ALU.add)
                        nc.vector.memset(
                            V_sb[st][:].rearrange("p (n c) -> p n c",
                                                  c=65)[:, :, 64:65],
                            1.0)

                v_group([0, 1, 2], 0)
                # Pair 0's Q proj + G tables + bounce slot in here: xT is
                # resident, PSUM has exactly 2 spare banks, and the gather
                # round-trip hides under V group 1 so head 0's attention can
                # start right after K p0.
                gh_dr = gdram.tile([48, NQS], LDT, tag="gh_dr",
                                   name="gh_dr")
                gw_dr = gdram.tile([63, NQS], LDT, tag="gw_dr",
                                   name="gw_dr")
                bwst = cpool.tile([32, NQS], LDT, tag="bwst", name="bwst")
                with (
                    tc.tile_pool(name="ps_p0", bufs=1, space="PSUM") as pp0,
                    tc.tile_pool(name="gst0", bufs=1) as gst0,
                ):
                    _pair_qg(nc, tc, 0, pp0, gst0, qh_all, bqk_sb, Wq_sb,
                             xT_sb, relh_sb, relw_sb, gh_dr, gw_dr, bwst,
                             LDT, AF)
                v_group([3, 4, 5], 1)
                v_group([6, 7], 2)

            with (
                tc.tile_pool(name="ps_qk", bufs=(1 if o2 else 2),
                             space="PSUM") as pq,
                tc.tile_pool(name="ps_g", bufs=1, space="PSUM") as pg,
                tc.tile_pool(name="gst", bufs=4) as gst,
                tc.tile_pool(name="ps_att", bufs=1, space="PSUM") as pa,
                tc.tile_pool(name="pu", bufs=10) as pu_pool,
                tc.tile_pool(name="rec", bufs=4) as rec_pool,
            ):
                # Software-pipelined pair loop: pair 0's Q/G ran inside
                # the V phase; each iteration runs K+unpermute+attention for
                # pair p and issues pair p+1's Q/G/bounce before attention so
                # the gather round-trip hides under the exp/PV stream.
                for p in range(6):
                    he, ho = 2 * p, 2 * p + 1
                    # K projection for the pair
                    for sh in range(2):
                        s0 = sh * 512
                        k_ps = pq.tile([128, 512], F32, tag="qk_ps",
                                       name="k_ps")
                        for kt in range(KT):
                            nc.tensor.matmul(
                                k_ps[:],
                                Wk_sb[kt][:, p * 128:(p + 1) * 128],
                                xT_sb[kt][:, s0:s0 + 512],
                                start=(kt == 0), stop=(kt == KT - 1))
                        nc.vector.tensor_scalar_add(
                            kh_all[0:64, he * HW + s0:he * HW + s0 + 512],
                            k_ps[0:64, :], bqk_sb[0:64, 2 * p + 1:2 * p + 2])
                        nc.vector.tensor_scalar_add(
                            kh_all[0:64, ho * HW + s0:ho * HW + s0 + 512],
                            k_ps[64:128, :], bqk_sb[64:128,
                                                    2 * p + 1:2 * p + 2])

                    for n in (he, ho):
                        nsl = slice(n * QS, (n + 1) * QS)
                        nc.vector.tensor_copy(
                            qh_all[96:128, nsl].rearrange(
                                "p (h w) -> p h w", w=WS),
                            bwst[:, nsl].rearrange("p (w h) -> p h w", h=QH))

                    if p < 5:
                        _pair_qg(nc, tc, p + 1, pq, gst, qh_all, bqk_sb,
                                 Wq_sb, xT_sb, relh_sb, relw_sb, gh_dr,
                                 gw_dr, bwst, LDT, AF)

                    # ---- attention for both heads of the pair ----
                    for n in (he, ho):
                        pu_tiles = []
                        for ktp in range(4):  # two k-tiles per psum tile
                            s_ps = pa.tile([128, 1024], F32, tag="s_ps",
                                           name="s_ps", bufs=2)
                            for j in range(2):
                                kt = 2 * ktp + j
                                nc.tensor.matmul(
                                    s_ps[:, j * 512:(j + 1) * 512],
                                    kh_all[:, n * HW + kt * 128:
                                           n * HW + (kt + 1) * 128],
                                    qh_all[:, n * QS:(n + 1) * QS],
                                    start=True, stop=True)
                            pu = pu_pool.tile([128, 1024], BF16, tag="pu",
                                              name="pu")
                            nc.scalar.activation(pu[:], s_ps[:], AF.Exp)
                            pu_tiles.append(pu)
                        o_ps = pa.tile([65, QS], F32, tag="o_ps",
                                       name="o_ps", bufs=(2 if o2 else 1))
                        for ktp in range(4):
                            for j in range(2):
                                kt = 2 * ktp + j
                                nc.tensor.matmul(
                                    o_ps[:],
                                    V_sb[kt][:, n * 65:n * 65 + 65],
                                    pu_tiles[ktp][:, j * 512:(j + 1) * 512],
                                    start=(kt == 0), stop=(kt == 7))
                        rec = rec_pool.tile([1, QS], F32, tag="rec",
                                            name="rec")
                        nc.vector.reciprocal(rec[:], o_ps[64:65, :])
                        rec_bc = rec_pool.tile([64, QS], F32, tag="rec_bc",
                                               name="rec_bc")
                        nc.gpsimd.partition_broadcast(rec_bc[:], rec[0:1, :])
                        nc.vector.tensor_tensor(
                            outT_sb[p][(n % 2) * 64:(n % 2 + 1) * 64, :],
                            o_ps[0:64, :],
                            rec_bc[:],
                            ALU.mult)

        # late constants for phase 4 (scalar queue, off the critical path)
        Wp_sb = []
        for p in range(6):
            t = cpool.tile([128, C], BF16, tag=f"wp{p}", name=f"wp{p}")
            nc.scalar.dma_start(t[:], Wp[p * 128:(p + 1) * 128, :])
            Wp_sb.append(t)
        bp_sb = cpool.tile([128, C], F32, tag="bp", name="bp")

        # ---- phase 4: output projection (+ bp) ----
        with (
            tc.tile_pool(name="ps_pr", bufs=2, space="PSUM") as pp_,
            tc.tile_pool(name="orow", bufs=2) as opool,
        ):
            bp_row = opool.tile([1, C], F32, tag="bp_row", name="bp_row")
            nc.gpsimd.dma_start(bp_row[:], bp2[:])
            nc.gpsimd.partition_broadcast(bp_sb[:], bp_row[0:1, :])
            for qt in range(4):
                qsl = slice(qt * 128, (qt + 1) * 128)
                pr = pp_.tile([128, C], F32, tag="pr", name="pr")
                for p in range(6):
                    for c0, c1 in ((0, 512), (512, 768)):
                        nc.tensor.matmul(
                            pr[:, c0:c1],
                            outT_sb[p][:, qsl],
                            Wp_sb[p][:, c0:c1],
                            start=(p == 0), stop=(p == 5))
                orow = opool.tile([128, C], BF16, tag="orow", name="orow")
                nc.vector.tensor_tensor(orow[:], pr[:], bp_sb[:], ALU.add)
                nc.sync.dma_start(out[qsl, :], orow[:])


def shard_inputs(hidden_states, Wq, bq, Wk, bk, Wv, bv, Wp, bp, rel_h, rel_w):
    """Build the 8 per-core input maps (host-side data movement only)."""
    f = np.float32
    em = np.zeros((64, HW), dtype=f)
    kk = np.arange(HW)
    em[31 - kk // WS, kk] = 1.0
    em[32 + 31 - kk % WS, kk] = 1.0
    rh8 = np.ascontiguousarray(8.0 * np.asarray(rel_h).astype(f).T)  # [64,63]
    rw8 = np.ascontiguousarray(8.0 * np.asarray(rel_w).astype(f).T)  # [64,63]

    wq = np.asarray(Wq).astype(f).astype(BF16NP)
    wk = np.asarray(Wk).astype(f).astype(BF16NP)
    wv = np.asarray(Wv).astype(f).astype(BF16NP)
    wp = np.asarray(Wp).astype(f).astype(BF16NP)
    bqk = np.ascontiguousarray(np.stack(
        [np.asarray(bq).astype(f) / 8.0, np.asarray(bk).astype(f)],
        axis=1))  # [C, 2]
    bvr = np.ascontiguousarray(np.asarray(bv).reshape(1, C).astype(f))
    bp2 = np.ascontiguousarray(np.asarray(bp).reshape(1, C).astype(f))

    in_maps = []
    for c in range(N_CORES):
        b, hp = c // 2, c % 2
        xTb = np.asarray(hidden_states)[b].reshape(HW, C).T.astype(f)
        perm = np.r_[hp * QS:(hp + 1) * QS, (1 - hp) * QS:(2 - hp) * QS]
        rel_c = np.zeros((64, 111), dtype=f)
        wdt = min(63 - hp * QH, 48)
        rel_c[:, :wdt] = rh8[:, hp * QH:hp * QH + wdt]
        rel_c[:, 48:111] = rw8
        ldt = BF16NP if LOGITS_BF16 else np.float32
        in_maps.append({
            "xT": np.ascontiguousarray(xTb[:, perm]).astype(BF16NP),
            "Wq": wq, "Wk": wk, "Wv": wv, "Wp": wp,
            "bqk": bqk, "bvr": bvr, "bp2": bp2,
            "rel": rel_c.astype(ldt),
            "em": np.ascontiguousarray(em[:, perm]).astype(ldt),
        })
    return in_maps


_NC_CACHE = {}


def get_program(loop_n=1):
    if loop_n not in _NC_CACHE:
        _NC_CACHE[loop_n] = build_program(loop_n=loop_n)
    return _NC_CACHE[loop_n]


def kernel(hidden_states, Wq, bq, Wk, bk, Wv, bv, Wp, bp, rel_h, rel_w):
    in_maps = shard_inputs(hidden_states, Wq, bq, Wk, bk, Wv, bv, Wp, bp,
                           rel_h, rel_w)
    nc = get_program()
    res = run_bass_kernel_spmd(nc, in_maps, list(range(N_CORES)))
    full = np.empty((B, HS, WS, C), dtype=np.float32)
    fr = full.reshape(B, HW, C)
    for c in range(N_CORES):
        b, hp = c // 2, c % 2
        fr[b, hp * QS:(hp + 1) * QS] = res.results[c]["out"].astype(
            np.float32)
    return full


# revision 41
# speedup vs baseline: 1.0173x; 1.0021x over previous
"""Trainium2 Bass kernel: multi-head attention with decomposed (rel_h + rel_w)
relative position bias.

Shapes (hardcoded): hidden_states (4, 32, 32, 768), NH=12, HD=64.

Sharding: sequence-parallel within each batch. Core c handles batch c//2 and
query rows [hp*512, hp*512+512) with hp = c%2, for ALL 12 heads. K and V are
computed fully (redundantly) by both cores of a pair, so every core owns its
512 output rows completely and no collective is needed.

Per-core trick inventory:
  - xT columns are host-permuted so the core's own 512 query columns come
    first; the selector table (em) is permuted identically, which keeps the
    program SPMD (no core-dependent constants). Attention is invariant to a
    consistent permutation of the key axis.
  - rel_h is host-shifted by hp*16 so the on-device gather offsets are
    core-independent.
  - relative bias is injected into the S^T matmul via basis rows: qh rows
    64..127 hold gathered G tables (G = q . rel), kh rows 64..127 hold a
    one-hot selector; contraction over all 128 rows yields q.k/8 + bias.
  - G tables bounce through DRAM (PSUM -> DRAM -> 3D-affine gather DMA)
    because the diagonal (shear) gather is only expressible on a flat tensor.
"""

import numpy as np
import ml_dtypes

import concourse.bass as bass
import concourse.bacc as bacc
import concourse.mybir as mybir
import concourse.tile as tile
from concourse.bass_utils import run_bass_kernel_spmd

B, HS, WS, C = 4, 32, 32, 768
NH, HD = 12, 64
HW = HS * WS          # 1024
N_CORES = 8
QS = HW // 2          # 512 query rows per core
QH = HS // 2          # 16 query h-rows per core
KT = C // 128         # 6 contraction tiles
NQS = NH * QS         # 6144
NHW = NH * HW         # 12288
F32 = mybir.dt.float32
F32R = mybir.dt.float32r
BF16 = mybir.dt.bfloat16
BF16NP = ml_dtypes.bfloat16


LOGITS_BF16 = True
O2_PSUM = True


def build_program(loop_n=1, logits_bf16=None, o2=None):
    if logits_bf16 is None:
        logits_bf16 = LOGITS_BF16
    if o2 is None:
        o2 = O2_PSUM
    nc = bacc.Bacc("TRN2", target_bir_lowering=False, debug=False,
                   num_devices=N_CORES)

    xT = nc.dram_tensor("xT", [C, HW], BF16, kind="ExternalInput").ap()
    Wq = nc.dram_tensor("Wq", [C, C], BF16, kind="ExternalInput").ap()
    Wk = nc.dram_tensor("Wk", [C, C], BF16, kind="ExternalInput").ap()
    Wv = nc.dram_tensor("Wv", [C, C], BF16, kind="ExternalInput").ap()
    Wp = nc.dram_tensor("Wp", [C, C], BF16, kind="ExternalInput").ap()
    bqk = nc.dram_tensor("bqk", [C, 2], F32, kind="ExternalInput").ap()
    bvr = nc.dram_tensor("bvr", [1, C], F32, kind="ExternalInput").ap()
    bp2 = nc.dram_tensor("bp2", [1, C], F32, kind="ExternalInput").ap()
    LDT = BF16 if logits_bf16 else F32R
    rel = nc.dram_tensor("rel", [64, 111], LDT, kind="ExternalInput").ap()
    em = nc.dram_tensor("em", [64, HW], LDT, kind="ExternalInput").ap()
    out = nc.dram_tensor("out", [QS, C], BF16, kind="ExternalOutput").ap()

    with tile.TileContext(nc) as tc:
        if loop_n > 1:
            with tc.For_i(0, loop_n):
                _body(nc, tc, xT, Wq, Wk, Wv, Wp, bqk, bvr, bp2,
                      rel, em, out, LDT, o2)
        else:
            _body(nc, tc, xT, Wq, Wk, Wv, Wp, bqk, bvr, bp2,
                  rel, em, out, LDT, o2)
    nc.compile()
    return nc


def _pair_qg(nc, tc, p, pq, gst, qh_all, bqk_sb, Wq_sb, xT_sb, relh_sb,
             relw_sb, gh_dr, gw_dr, bwst, LDT, AF):
    """Q projection + G tables + DRAM bounce + gathers for head pair p."""
    he, ho = 2 * p, 2 * p + 1
    q_ps = pq.tile([128, QS], F32, tag="qk_ps", name="q_ps")
    for kt in range(KT):
        nc.tensor.matmul(
            q_ps[:],
            Wq_sb[kt][:, p * 128:(p + 1) * 128],
            xT_sb[kt][:, 0:QS],
            start=(kt == 0), stop=(kt == KT - 1))
    # qs = q/8 + bq/8 (bqk col 2p holds bq/8) on Act
    nc.scalar.activation(
        qh_all[0:64, he * QS:(he + 1) * QS],
        q_ps[0:64, :], AF.Identity,
        bias=bqk_sb[0:64, 2 * p:2 * p + 1], scale=0.125)
    nc.scalar.activation(
        qh_all[0:64, ho * QS:(ho + 1) * QS],
        q_ps[64:128, :], AF.Identity,
        bias=bqk_sb[64:128, 2 * p:2 * p + 1], scale=0.125)

    # G tables + DRAM bounce (SP engine owns every bounce DMA trigger so
    # compute engines never head-block)
    for n in (he, ho):
        nsl = slice(n * QS, (n + 1) * QS)
        gh_ps = pq.tile([48, QS], F32, tag="g_ps", name="gh_ps")
        nc.tensor.matmul(
            gh_ps[:], relh_sb, qh_all[0:64, nsl],
            start=True, stop=True, tile_position=(0, 0))
        gh_sb = gst.tile([48, QS], LDT, tag="gh_sb", name="gh_sb")
        nc.scalar.copy(gh_sb[:], gh_ps[:])
        nc.sync.dma_start(gh_dr[:, nsl], gh_sb[:])
        gw_ps = pq.tile([63, QS], F32, tag="g_ps", name="gw_ps")
        nc.tensor.matmul(
            gw_ps[:], relw_sb,
            qh_all[0:64, nsl].rearrange("p (h w) -> p w h", w=WS),
            start=True, stop=True, tile_position=(0, 0))
        gw_sb = gst.tile([63, QS], LDT, tag="gw_sb", name="gw_sb")
        nc.vector.tensor_copy(gw_sb[:], gw_ps[:])
        nc.sync.dma_start(gw_dr[:, nsl], gw_sb[:])
    # gathers: shear G[h'+r] / G[w+r] via flat-DRAM 3D APs. bh lands
    # directly in qh rows 64-95; bw lands w-major in bwst.
    for n in (he, ho):
        nsl = slice(n * QS, (n + 1) * QS)
        dst_h = qh_all[64:96, nsl].rearrange("p (h w) -> p h w", w=WS)
        src_h = bass.AP(tensor=gh_dr[:].tensor, offset=n * QS,
                        ap=[[NQS, 32], [NQS + WS, QH], [1, WS]])
        nc.sync.dma_start(dst_h, src_h)
        dst_w = bwst[:, nsl].rearrange("p (w h) -> p w h", h=QH)
        src_w = bass.AP(tensor=gw_dr[:].tensor, offset=n * QS,
                        ap=[[NQS, 32], [NQS + QH, WS], [1, QH]])
        nc.sync.dma_start(dst_w, src_w)


def _body(nc, tc, xT, Wq, Wk, Wv, Wp, bqk, bvr, bp2, rel, em, out,
          LDT, o2=False):
    AF = mybir.ActivationFunctionType
    ALU = mybir.AluOpType

    with (
        tc.tile_pool(name="const", bufs=1) as cpool,
        tc.tile_pool(name="work", bufs=1) as wpool,
        tc.tile_pool(name="gdram", bufs=1, space="DRAM") as gdram,
    ):
        rel_sb = cpool.tile([64, 111], LDT, tag="rel", name="rel")
        nc.scalar.dma_start(rel_sb[:], rel[:])
        relh_sb = rel_sb[:, 0:48]
        relw_sb = rel_sb[:, 48:111]
        bvr_sb = cpool.tile([128, C], F32, tag="bvr", name="bvr")

        # per-head stacked tensors:
        #   qh_all rows: 0-63 qs^T, 64-95 bh basis, 96-127 bw basis
        #   kh_all rows: 0-63 k^T,  64-127 selector (em)
        qh_all = wpool.tile([128, NQS], LDT, tag="qh", name="qh")
        kh_all = wpool.tile([128, NHW], LDT, tag="kh", name="kh")
        V_sb = [wpool.tile([128, NH * 65], BF16, tag=f"v{st}", name=f"v{st}")
                for st in range(8)]
        outT_sb = [wpool.tile([128, QS], BF16, tag=f"oT{p}", name=f"oT{p}")
                   for p in range(6)]


        with tc.tile_pool(name="ph1", bufs=1) as ph1:
            bvr_row = ph1.tile([1, C], F32, tag="bvr_row", name="bvr_row")
            nc.gpsimd.dma_start(bvr_row[:], bvr[:])
            nc.gpsimd.partition_broadcast(bvr_sb[:], bvr_row[0:1, :])
            # selector rows (shared across q; columns follow the host
            # k-perm). One DMA for head 0, then on-chip Pool copies for the
            # other 11 heads (saves ~2.8MB of DMA; the copies sit behind the
            # bvr broadcast so V-assembly is never blocked).
            nc.gpsimd.dma_start(kh_all[64:128, 0:HW], em[0:64, :])
            for n in range(1, NH):
                nc.gpsimd.tensor_copy(kh_all[64:128, n * HW:(n + 1) * HW],
                                      kh_all[64:128, 0:HW])
            xT_sb, Wv_sb, Wq_sb, Wk_sb = [], [], [], []
            for kt in range(KT):
                t = ph1.tile([128, HW], BF16, tag=f"xT{kt}", name=f"xT{kt}")
                if kt == 0:
                    # split so the first V matmul's dependency is tiny
                    nc.sync.dma_start(t[:, 0:384],
                                      xT[0:128, 0:384])
                    nc.sync.dma_start(t[:, 384:HW],
                                      xT[0:128, 384:HW])
                else:
                    nc.sync.dma_start(t[:], xT[kt * 128:(kt + 1) * 128, :])
                xT_sb.append(t)
                t = ph1.tile([128, C], BF16, tag=f"wv{kt}", name=f"wv{kt}")
                if kt == 0:
                    nc.sync.dma_start(t[:, 0:512], Wv[0:128, 0:512])
                    nc.sync.dma_start(t[:, 512:C], Wv[0:128, 512:C])
                else:
                    nc.sync.dma_start(t[:], Wv[kt * 128:(kt + 1) * 128, :])
                Wv_sb.append(t)
            bqk_sb = ph1.tile([128, 12], F32, tag="bqk", name="bqk")
            nc.scalar.dma_start(
                bqk_sb[:].rearrange("p (a j) -> p a j", j=2),
                bass.AP(tensor=bqk[:].tensor, offset=0,
                        ap=[[2, 128], [256, 6], [1, 2]]))
            for kt in range(KT):
                t = ph1.tile([128, C], BF16, tag=f"wq{kt}", name=f"wq{kt}")
                nc.sync.dma_start(t[:], Wq[kt * 128:(kt + 1) * 128, :])
                Wq_sb.append(t)
            for kt in range(KT):
                t = ph1.tile([128, C], BF16, tag=f"wk{kt}", name=f"wk{kt}")
                nc.gpsimd.dma_start(t[:], Wk[kt * 128:(kt + 1) * 128, :])
                Wk_sb.append(t)
            # Wp prefetch on the same idle queue: if left on the Act queue
            # its triggers sit behind all 48 exps and the output projection
            # stalls ~3us waiting for the weights.
            Wp_sb = []
            for p in range(6):
                t = cpool.tile([128, C], BF16, tag=f"wp{p}", name=f"wp{p}")
                nc.gpsimd.dma_start(t[:], Wp[p * 128:(p + 1) * 128, :])
                Wp_sb.append(t)

            # ---- phase 1a: V (kt-outer so PE starts after ~0.5MB of DMA),
            # two groups of 4 row-tiles to fit PSUM ----
            with tc.tile_pool(name="ps_v", bufs=1, space="PSUM") as pv:
                def v_group(sts, g):
                    v_ps = [pv.tile([128, C], F32, tag=f"v_ps{i}",
                                    name=f"v_ps{g}{i}")
                            for i in range(len(sts))]
                    for kt in range(KT):
                        for i, st in enumerate(sts):
                            for c0, c1 in ((0, 512), (512, 768)):
                                nc.tensor.matmul(
                                    v_ps[i][:, c0:c1],
                                    xT_sb[kt][:, st * 128:(st + 1) * 128],
                                    Wv_sb[kt][:, c0:c1],
                                    start=(kt == 0), stop=(kt == KT - 1))
                    for i, st in enumerate(sts):
                        nc.vector.tensor_tensor(
                            V_sb[st][:].rearrange("p (n c) -> p n c",
                                                  c=65)[:, :, 0:64],
                            v_ps[i][:].rearrange("p (n c) -> p n c", c=64),
                            bvr_sb[:].rearrange("p (n c) -> p n c", c=64),
                            ALU.add)
                        nc.vector.memset(
                            V_sb[st][:].rearrange("p (n c) -> p n c",
                                                  c=65)[:, :, 64:65],
                            1.0)

                v_group([0, 1, 2], 0)
                # Pair 0's Q proj + G tables + bounce slot in here: xT is
                # resident, PSUM has exactly 2 spare banks, and the gather
                # round-trip hides under V group 1 so head 0's attention can
                # start right after K p0.
                gh_dr = gdram.tile([48, NQS], LDT, tag="gh_dr",
                                   name="gh_dr")
                gw_dr = gdram.tile([63, NQS], LDT, tag="gw_dr",
                                   name="gw_dr")
                bwst = cpool.tile([32, NQS], LDT, tag="bwst", name="bwst")
                with (
                    tc.tile_pool(name="ps_p0", bufs=1, space="PSUM") as pp0,
                    tc.tile_pool(name="gst0", bufs=1) as gst0,
                ):
                    _pair_qg(nc, tc, 0, pp0, gst0, qh_all, bqk_sb, Wq_sb,
                             xT_sb, relh_sb, relw_sb, gh_dr, gw_dr, bwst,
                             LDT, AF)
                v_group([3, 4, 5], 1)
                v_group([6, 7], 2)

            with (
                tc.tile_pool(name="ps_qk", bufs=(1 if o2 else 2),
                             space="PSUM") as pq,
                tc.tile_pool(name="ps_g", bufs=1, space="PSUM") as pg,
                tc.tile_pool(name="gst", bufs=6) as gst,
                tc.tile_pool(name="ps_att", bufs=1, space="PSUM") as pa,
                tc.tile_pool(name="pu", bufs=12) as pu_pool,
                tc.tile_pool(name="rec", bufs=4) as rec_pool,
            ):
                # Software-pipelined pair loop: pair 0's Q/G ran inside
                # the V phase; each iteration runs K+unpermute+attention for
                # pair p and issues pair p+1's Q/G/bounce before attention so
                # the gather round-trip hides under the exp/PV stream.
                for p in range(6):
                    he, ho = 2 * p, 2 * p + 1
                    # K projection for the pair
                    for sh in range(2):
                        s0 = sh * 512
                        k_ps = pq.tile([128, 512], F32, tag="qk_ps",
                                       name="k_ps")
                        for kt in range(KT):
                            nc.tensor.matmul(
                                k_ps[:],
                                Wk_sb[kt][:, p * 128:(p + 1) * 128],
                                xT_sb[kt][:, s0:s0 + 512],
                                start=(kt == 0), stop=(kt == KT - 1))
                        nc.vector.tensor_scalar_add(
                            kh_all[0:64, he * HW + s0:he * HW + s0 + 512],
                            k_ps[0:64, :], bqk_sb[0:64, 2 * p + 1:2 * p + 2])
                        nc.vector.tensor_scalar_add(
                            kh_all[0:64, ho * HW + s0:ho * HW + s0 + 512],
                            k_ps[64:128, :], bqk_sb[64:128,
                                                    2 * p + 1:2 * p + 2])

                    for n in (he, ho):
                        nsl = slice(n * QS, (n + 1) * QS)
                        nc.vector.tensor_copy(
                            qh_all[96:128, nsl].rearrange(
                                "p (h w) -> p h w", w=WS),
                            bwst[:, nsl].rearrange("p (w h) -> p h w", h=QH))

                    if p < 5:
                        _pair_qg(nc, tc, p + 1, pq, gst, qh_all, bqk_sb,
                                 Wq_sb, xT_sb, relh_sb, relw_sb, gh_dr,
                                 gw_dr, bwst, LDT, AF)

                    # ---- attention for both heads of the pair ----
                    for n in (he, ho):
                        pu_tiles = []
                        for ktp in range(4):  # two k-tiles per psum tile
                            s_ps = pa.tile([128, 1024], F32, tag="s_ps",
                                           name="s_ps", bufs=2)
                            for j in range(2):
                                kt = 2 * ktp + j
                                nc.tensor.matmul(
                                    s_ps[:, j * 512:(j + 1) * 512],
                                    kh_all[:, n * HW + kt * 128:
                                           n * HW + (kt + 1) * 128],
                                    qh_all[:, n * QS:(n + 1) * QS],
                                    start=True, stop=True)
                            pu = pu_pool.tile([128, 1024], BF16, tag="pu",
                                              name="pu")
                            nc.scalar.activation(pu[:], s_ps[:], AF.Exp)
                            pu_tiles.append(pu)
                        o_ps = pa.tile([65, QS], F32, tag="o_ps",
                                       name="o_ps", bufs=(2 if o2 else 1))
                        for ktp in range(4):
                            for j in range(2):
                                kt = 2 * ktp + j
                                nc.tensor.matmul(
                                    o_ps[:],
                                    V_sb[kt][:, n * 65:n * 65 + 65],
                                    pu_tiles[ktp][:, j * 512:(j + 1) * 512],
                                    start=(kt == 0), stop=(kt == 7))
                        rec = rec_pool.tile([1, QS], F32, tag="rec",
                                            name="rec")
                        nc.vector.reciprocal(rec[:], o_ps[64:65, :])
                        rec_bc = rec_pool.tile([64, QS], F32, tag="rec_bc",
                                               name="rec_bc")
                        nc.gpsimd.partition_broadcast(rec_bc[:], rec[0:1, :])
                        nc.vector.tensor_tensor(
                            outT_sb[p][(n % 2) * 64:(n % 2 + 1) * 64, :],
                            o_ps[0:64, :],
                            rec_bc[:],
                            ALU.mult)

        # (Wp was prefetched on the Pool queue inside ph1.)
        bp_sb = cpool.tile([128, C], F32, tag="bp", name="bp")

        # ---- phase 4: output projection (+ bp) ----
        with (
            tc.tile_pool(name="ps_pr", bufs=2, space="PSUM") as pp_,
            tc.tile_pool(name="orow", bufs=2) as opool,
        ):
            bp_row = opool.tile([1, C], F32, tag="bp_row", name="bp_row")
            nc.gpsimd.dma_start(bp_row[:], bp2[:])
            nc.gpsimd.partition_broadcast(bp_sb[:], bp_row[0:1, :])
            for qt in range(4):
                qsl = slice(qt * 128, (qt + 1) * 128)
                pr = pp_.tile([128, C], F32, tag="pr", name="pr")
                for p in range(6):
                    for c0, c1 in ((0, 512), (512, 768)):
                        nc.tensor.matmul(
                            pr[:, c0:c1],
                            outT_sb[p][:, qsl],
                            Wp_sb[p][:, c0:c1],
                            start=(p == 0), stop=(p == 5))
                orow = opool.tile([128, C], BF16, tag="orow", name="orow")
                nc.vector.tensor_tensor(orow[:], pr[:], bp_sb[:], ALU.add)
                nc.sync.dma_start(out[qsl, :], orow[:])


def shard_inputs(hidden_states, Wq, bq, Wk, bk, Wv, bv, Wp, bp, rel_h, rel_w):
    """Build the 8 per-core input maps (host-side data movement only)."""
    f = np.float32
    em = np.zeros((64, HW), dtype=f)
    kk = np.arange(HW)
    em[31 - kk // WS, kk] = 1.0
    em[32 + 31 - kk % WS, kk] = 1.0
    rh8 = np.ascontiguousarray(8.0 * np.asarray(rel_h).astype(f).T)  # [64,63]
    rw8 = np.ascontiguousarray(8.0 * np.asarray(rel_w).astype(f).T)  # [64,63]

    wq = np.asarray(Wq).astype(f).astype(BF16NP)
    wk = np.asarray(Wk).astype(f).astype(BF16NP)
    wv = np.asarray(Wv).astype(f).astype(BF16NP)
    wp = np.asarray(Wp).astype(f).astype(BF16NP)
    bqk = np.ascontiguousarray(np.stack(
        [np.asarray(bq).astype(f) / 8.0, np.asarray(bk).astype(f)],
        axis=1))  # [C, 2]
    bvr = np.ascontiguousarray(np.asarray(bv).reshape(1, C).astype(f))
    bp2 = np.ascontiguousarray(np.asarray(bp).reshape(1, C).astype(f))

    in_maps = []
    for c in range(N_CORES):
        b, hp = c // 2, c % 2
        xTb = np.asarray(hidden_states)[b].reshape(HW, C).T.astype(f)
        perm = np.r_[hp * QS:(hp + 1) * QS, (1 - hp) * QS:(2 - hp) * QS]
        rel_c = np.zeros((64, 111), dtype=f)
        wdt = min(63 - hp * QH, 48)
        rel_c[:, :wdt] = rh8[:, hp * QH:hp * QH + wdt]
        rel_c[:, 48:111] = rw8
        ldt = BF16NP if LOGITS_BF16 else np.float32
        in_maps.append({
            "xT": np.ascontiguousarray(xTb[:, perm]).astype(BF16NP),
            "Wq": wq, "Wk": wk, "Wv": wv, "Wp": wp,
            "bqk": bqk, "bvr": bvr, "bp2": bp2,
            "rel": rel_c.astype(ldt),
            "em": np.ascontiguousarray(em[:, perm]).astype(ldt),
        })
    return in_maps


_NC_CACHE = {}


def get_program(loop_n=1):
    if loop_n not in _NC_CACHE:
        _NC_CACHE[loop_n] = build_program(loop_n=loop_n)
    return _NC_CACHE[loop_n]


def kernel(hidden_states, Wq, bq, Wk, bk, Wv, bv, Wp, bp, rel_h, rel_w):
    in_maps = shard_inputs(hidden_states, Wq, bq, Wk, bk, Wv, bv, Wp, bp,
                           rel_h, rel_w)
    nc = get_program()
    res = run_bass_kernel_spmd(nc, in_maps, list(range(N_CORES)))
    full = np.empty((B, HS, WS, C), dtype=np.float32)
    fr = full.reshape(B, HW, C)
    for c in range(N_CORES):
        b, hp = c // 2, c % 2
        fr[b, hp * QS:(hp + 1) * QS] = res.results[c]["out"].astype(
            np.float32)
    return full
